# revision 43
# baseline (speedup 1.0000x reference)
"""Trainium2 Bass kernel for nn_CrossAttention_61890478735686.

Math per (batch n, unit u):
    q = query[n] viewed [c=256, hw=256];  raw DRAM layout [hw, c] = q^T
    k = v = value[n] same.
    qW = q @ Wq[u]   [256, 64]
    kW = k @ Wk[u]   [256, 64]
    dot = qW @ kW^T  [256, 256];  attn = softmax(dot/16, axis=-1)
    vW = k @ Wv[u]   [256, 9]
    out = attn @ vW  [256, 9] -> output[n, kh, kw, c, u], m = 3*kh+kw

Kernel dataflow (everything transposed so softmax reduction is the
contraction axis of the final matmul):
    qWT[q, c]  = Wq[u]^T @ q^T      (lhsT = Wq chunks, rhs = raw query)
    kWT[q, c]  = Wk[u]^T @ k^T
    dotT[d, c] = kWT^T-contraction over q (lhsT = kWT cols, rhs = qWT)
    ET = exp(dotT / 16)             (ACT, PSUM -> SBUF)
    unnorm[m, c] = vW_aug^T @ ET    (vW augmented with a ones column ->
                                     row 9 = softmax denominator S[c])
    host: out = unnorm[:9] / unnorm[9]

Sharding: tensor-parallel over units. Core i gets units 16i..16i+16 and
all 16 batches (256 (n,u) pairs per core).

Execution: a module-cached jitted 8-core SPMD executor (built once per
process). Weights upload core-sharded; query/value upload batch-sharded
and are replicated on device by a separate jitted reshard (the bass_jit
compile hook cannot host collectives). The NEFF's output buffers are
device-resident scratch: the kernel overwrites every element each run.

Main loop is g-outer/n-inner with input DMAs ordered by first use on
the SP queue, so group 0 starts ~3us in.  The vW matmuls run in bf16
(one-time Pool-engine conversions of v/Wv): their 36-wide output pays
f32r's 4x cycles-per-row penalty but streams at 1x in bf16.  The ones
column is memset on device (no input tensor).  PSUM-sourced copies
alternate DVE/Act to balance engine load.  CoreSim device time ~186us
with PE (the critical engine) at its 169.5us dataflow floor, >90%
dense from ~4us to ~181us.
"""

import sys

if "/opt/trn_rl_repo" not in sys.path:
    sys.path.insert(0, "/opt/trn_rl_repo")

import numpy as np

import concourse.bass as bass
import concourse.tile as tile
from concourse import mybir

F32 = mybir.dt.float32
F32R = mybir.dt.float32r
BF16 = mybir.dt.bfloat16

N_CORES = 8
NB = 16          # batches
UPC = 16         # units per core
C = 256          # channels
HW = 256         # h*w (contraction dim of the projections)
QK = 64          # qk_dim
M = 9            # kernel_len
MA = 10          # M + ones column
SCALE = 1.0 / 16.0

USE_F32R = True
MMDT = F32R if USE_F32R else F32
ETDT = MMDT


def split_multiwait_drains(nc):
    """This walrus build cannot codegen instructions carrying >1 sem wait
    (CoreV3GenImpl setupSyncWait: 'Too many sync wait commands').  Hoist
    all but the last wait into single-wait NOPs preceding the instruction
    on the same engine — semantically identical (the sequencer stalls on
    each in turn)."""
    for f in nc.m.functions:
        for bb in f.blocks:
            new_insts = []
            for inst in bb.instructions:
                si = getattr(inst, "sync_info", None)
                if si is not None and len(si.on_wait) > 1:
                    waits = list(si.on_wait)
                    for j, w in enumerate(waits[:-1]):
                        nop = mybir.InstNoOp(
                            name=f"{inst.name}-wsplit{j}",
                            engine=inst.engine,
                            ins=[],
                            outs=[],
                            sync_info=mybir.SyncInfo(on_wait=[w], on_update=[]),
                        )
                        new_insts.append(nop)
                    si.on_wait = [waits[-1]]
                new_insts.append(inst)
            bb.instructions = new_insts


def build_nc():
    nc = bass.Bass()

    q_d = nc.dram_tensor("query", [NB, HW, C], MMDT, kind="ExternalInput")
    v_d = nc.dram_tensor("value", [NB, HW, C], MMDT, kind="ExternalInput")
    wq_d = nc.dram_tensor("query_w", [UPC, HW, QK], MMDT, kind="ExternalInput")
    wk_d = nc.dram_tensor("key_w", [UPC, HW, QK], MMDT, kind="ExternalInput")
    wv_d = nc.dram_tensor("value_w", [UPC, HW, M], MMDT, kind="ExternalInput")
    out_d = nc.dram_tensor("out", [NB, UPC, MA, C], F32, kind="ExternalOutput")

    with tile.TileContext(nc) as tc:
        with (
            tc.tile_pool(name="persist", bufs=1) as persist,
            tc.tile_pool(name="kqp", bufs=3) as kqp,
            tc.tile_pool(name="etp", bufs=4) as etp,
            tc.tile_pool(name="augp", bufs=3) as augp,
            tc.tile_pool(name="outp", bufs=3) as outp,
            tc.tile_pool(name="ps_qk", bufs=2, space="PSUM") as ps_qk,
            tc.tile_pool(name="ps_dot", bufs=3, space="PSUM") as ps_dot,
            tc.tile_pool(name="ps_vw", bufs=1, space="PSUM") as ps_vw,
            tc.tile_pool(name="ps_out", bufs=1, space="PSUM") as ps_out,
        ):
            # ---- persistent inputs --------------------------------------
            # hw row 2p+k lives at [partition p, chunk k]: q/v partitions
            # read contiguous 2-row (2KB) runs from DRAM.  All contractions
            # over hw are permutation-invariant as long as q/v and the
            # weights use the same hw layout.
            # q_sb/v_sb: [p=128, n, k, c]
            q_sb = persist.tile([128, NB, 2, C], MMDT)
            v_sb = persist.tile([128, NB, 2, C], MMDT)
            # wq/wk: [p, pair, k, u2, qk]; lhsT slice [:, pr, k] contiguous
            wq_sb = persist.tile([128, UPC // 2, 2, 2, QK], MMDT)
            wk_sb = persist.tile([128, UPC // 2, 2, 2, QK], MMDT)
            # wv: [p, k, u, m]
            wv_sb = persist.tile([128, 2, UPC, M], MMDT)
            ones_sb = persist.tile([128, 1], F32)
            # bf16 shadows of v / wv: the vW matmul's 36-wide output pays
            # f32r's 4x cycles-per-row penalty (<256 free) but runs at
            # 1x in bf16.  One-time conversions on the Pool engine.
            v16 = persist.tile([128, NB, 2, C], BF16)
            wv16 = persist.tile([128, 2, UPC, M], BF16)

            # DMA issue order is latency-critical: the g-outer main loop
            # needs (ones, wv, wq/wk pair 0..1, q/v batch 0) before group 0
            # starts.  SP carries those plus the q/v stream; the remaining
            # weight pairs go on the Activation queue engine, which is idle
            # until the first exp several us in.
            def load_w(eng, w_sb, w_d, pr):
                eng.dma_start(
                    out=w_sb[:, pr],
                    in_=w_d[2 * pr : 2 * pr + 2].rearrange(
                        "u (p k) q -> p k u q", p=128
                    ),
                )

            # group 0 / batch 0 critical path first
            load_w(nc.sync, wq_sb, wq_d, 0)
            load_w(nc.sync, wk_sb, wk_d, 0)
            nc.sync.dma_start(
                out=q_sb[:, 0], in_=q_d[0].rearrange("(p k) c -> p k c", p=128)
            )
            nc.sync.dma_start(
                out=v_sb[:, 0], in_=v_d[0].rearrange("(p k) c -> p k c", p=128)
            )
            nc.sync.dma_start(
                out=wv_sb[:], in_=wv_d.rearrange("u (p k) m -> p k u m", p=128)
            )
            load_w(nc.sync, wq_sb, wq_d, 1)
            load_w(nc.sync, wk_sb, wk_d, 1)
            for n in range(1, NB):
                nc.sync.dma_start(
                    out=q_sb[:, n], in_=q_d[n].rearrange("(p k) c -> p k c", p=128)
                )
                nc.sync.dma_start(
                    out=v_sb[:, n], in_=v_d[n].rearrange("(p k) c -> p k c", p=128)
                )
            # later weight pairs: pair pr is first needed at the g=pr//2
            # sweep (~46us * pr//2 in), so queueing them after the q/v
            # stream on SP still arrives early; keeps Act free for exp
            for pr in range(2, UPC // 2):
                load_w(nc.sync, wq_sb, wq_d, pr)
                load_w(nc.sync, wk_sb, wk_d, pr)

            # ---- main loop (final stage software-pipelined by 1 group) ---
            def emit_final(st):
                et_tiles, vw_aug, n, g = st
                # final: unnorm outT per unit, col-packed 4 units/bank
                psum_out = ps_out.tile([16, 4, C], F32, name="psum_out")
                for u4 in range(4):
                    sp, uu = divmod(u4, 2)
                    for j in range(2):
                        nc.tensor.matmul(
                            psum_out[0:MA, u4],
                            vw_aug[:, j, u4],
                            et_tiles[sp][:, uu, j],
                            start=(j == 0),
                            stop=(j == 1),
                        )
                out_sb = outp.tile([16, 4, C], F32, name="out_sb")
                nc.vector.tensor_copy(out_sb[0:MA], psum_out[0:MA])
                nc.gpsimd.dma_start(
                    out=out_d[n, 4 * g : 4 * g + 4].rearrange("u m c -> m u c"),
                    in_=out_sb[0:MA],
                )

            nc.vector.memset(ones_sb[:], 1.0)
            nc.gpsimd.tensor_copy(wv16[:], wv_sb[:])

            pending = None
            for g in range(UPC // 4):  # group of 4 units
                for n in range(NB):
                    if g == 0:
                        # lazy per-batch f32->bf16 conversion, first use
                        nc.gpsimd.tensor_copy(v16[:, n], v_sb[:, n])
                    # vW for the 4 units: psum_vw[:, j, u4, m], j = ch chunk
                    psum_vw = ps_vw.tile([128, 2, 4, M], F32, name="psum_vw")
                    for j in range(2):
                        for k in range(2):
                            nc.tensor.matmul(
                                psum_vw[:, j],
                                v16[:, n, k, 128 * j : 128 * (j + 1)],
                                wv16[:, k, 4 * g : 4 * g + 4],
                                start=(k == 0),
                                stop=(k == 1),
                            )
                    # augmented [p, j, u4, 10]: col 9 = 1.0 (softmax denom row)
                    # PSUM-sourced copies can only run on DVE or Act (walrus
                    # rejects Pool); alternate to balance the two engines.
                    # Copy shares Act's exp table, so no table reloads.
                    vw_aug = augp.tile([128, 2, 4, MA], MMDT, name="vw_aug")
                    if (g * NB + n) % 2 == 0:
                        nc.vector.tensor_copy(vw_aug[:, :, :, 0:M], psum_vw[:])
                    else:
                        nc.scalar.copy(vw_aug[:, :, :, 0:M], psum_vw[:])
                    nc.gpsimd.tensor_copy(
                        vw_aug[:, :, :, M:MA], ones_sb.to_broadcast([128, 2, 4, 1])
                    )

                    et_tiles = []
                    for sp in range(2):  # sub-pair of units
                        pr = 2 * g + sp
                        # qWT/kWT 2 units stacked: psum_qk[:,0]=q, [:,1]=k
                        psum_qk = ps_qk.tile([128, 2, C], F32, name="psum_qk")
                        for k in range(2):
                            nc.tensor.matmul(
                                psum_qk[:, 0],
                                wq_sb[:, pr, k],
                                q_sb[:, n, k],
                                start=(k == 0),
                                stop=(k == 1),
                            )
                        for k in range(2):
                            nc.tensor.matmul(
                                psum_qk[:, 1],
                                wk_sb[:, pr, k],
                                v_sb[:, n, k],
                                start=(k == 0),
                                stop=(k == 1),
                            )
                        kq_sb = kqp.tile([128, 2, C], MMDT, name="kq_sb")
                        nc.vector.tensor_copy(kq_sb[:], psum_qk[:])

                        # dotT: [d' chunk j, c] per unit uu; one 1-bank PSUM
                        # tile per unit so exp(uu=0) overlaps matmul(uu=1)
                        et_sb = etp.tile([128, 2, 2, C], ETDT, name="et_sb")
                        for uu in range(2):
                            psum_dot = ps_dot.tile(
                                [128, 2, C], F32, name="psum_dot"
                            )  # [p, j, c]
                            for j in range(2):
                                nc.tensor.matmul(
                                    psum_dot[:, j],
                                    kq_sb[
                                        64 * uu : 64 * uu + 64,
                                        1,
                                        128 * j : 128 * (j + 1),
                                    ],
                                    kq_sb[64 * uu : 64 * uu + 64, 0],
                                    start=True,
                                    stop=True,
                                )
                            nc.scalar.activation(
                                out=et_sb[:, uu],
                                in_=psum_dot[:],
                                func=mybir.ActivationFunctionType.Exp,
                                scale=SCALE,
                            )
                        et_tiles.append(et_sb)

                    if pending is not None:
                        emit_final(pending)
                    pending = (et_tiles, vw_aug, n, g)
            emit_final(pending)

    split_multiwait_drains(nc)
    return nc


_NC_CACHE = None


def _get_nc():
    global _NC_CACHE
    if _NC_CACHE is None:
        _NC_CACHE = build_nc()
    return _NC_CACHE


_EXEC_CACHE = None


def _get_executor():
    """Build (once) a jitted 8-core SPMD executor for the bass module.

    Unlike run_bass_kernel_spmd, this is traced/compiled a single time;
    repeat kernel() calls only pay input upload + execute + download.
    The NEFF's output buffers are device-resident scratch (the kernel
    overwrites every element on each run, so they are never re-zeroed).
    """
    global _EXEC_CACHE
    if _EXEC_CACHE is not None:
        return _EXEC_CACHE

    import jax
    from jax.sharding import Mesh, PartitionSpec
    from jax.experimental.shard_map import shard_map
    from concourse import bass2jax
    from concourse.bass2jax import _bass_exec_p, install_neuronx_cc_hook

    install_neuronx_cc_hook()
    nc = _get_nc()

    pname = nc.partition_id_tensor.name if nc.partition_id_tensor else None
    in_names, out_names, out_avals, out_shapes = [], [], [], []
    for alloc in nc.m.functions[0].allocations:
        if not isinstance(alloc, mybir.MemoryLocationSet):
            continue
        name = alloc.memorylocations[0].name
        if alloc.kind == "ExternalInput":
            if name != pname:
                in_names.append(name)
        elif alloc.kind == "ExternalOutput":
            out_names.append(name)
            shape = tuple(alloc.tensor_shape)
            dtype = mybir.dt.np(alloc.dtype)
            out_avals.append(jax.core.ShapedArray(shape, dtype))
            out_shapes.append((shape, dtype))
    n_params = len(in_names)
    all_in_names = in_names + out_names

    def _body(*args):
        operands = list(args)
        if pname is not None:
            operands.append(bass2jax.partition_id_tensor())
        outs = _bass_exec_p.bind(
            *operands,
            out_avals=tuple(out_avals),
            in_names=tuple(all_in_names + ([pname] if pname else [])),
            out_names=tuple(out_names),
            lowering_input_output_aliases=(),
            sim_require_finite=True,
            sim_require_nnan=True,
            nc=nc,
        )
        return tuple(outs)

    devices = jax.devices()[:N_CORES]
    mesh = Mesh(np.asarray(devices), ("core",))
    nouts = len(out_names)
    # query/value are replicated across cores (tensor-parallel over units);
    # shard_map declares them P() so batch-sharded uploads are all-gathered
    # on device instead of being pushed over the wire 8x.
    REPL = ("query", "value")
    in_specs = tuple(
        PartitionSpec() if nm in REPL else PartitionSpec("core")
        for nm in in_names
    ) + (PartitionSpec("core"),) * nouts
    sharded = jax.jit(
        shard_map(
            _body,
            mesh=mesh,
            in_specs=in_specs,
            out_specs=(PartitionSpec("core"),) * nouts,
            check_rep=False,
        ),
        keep_unused=True,
    )
    zo_dev = [
        jax.device_put(np.zeros((N_CORES * s[0], *s[1:]), d))
        for s, d in out_shapes
    ]
    from jax.sharding import NamedSharding

    core_sharding = NamedSharding(mesh, PartitionSpec("core"))
    repl_sharding = NamedSharding(mesh, PartitionSpec())
    # On-device batch-sharded -> replicated reshard (plain XLA program;
    # the bass_jit compile hook cannot host the all-gather itself).
    replicate = jax.jit(lambda x: x, out_shardings=repl_sharding)

    _EXEC_CACHE = {
        "jax": jax,
        "sharded": sharded,
        "in_names": in_names,
        "zo_dev": zo_dev,
        "out_shapes": out_shapes,
        "core_sharding": core_sharding,
        "repl_sharding": repl_sharding,
        "replicate": replicate,
        "REPL": REPL,
    }
    return _EXEC_CACHE


def make_global_inputs(query, value, query_w, key_w, value_w):
    """Global (all-cores) input arrays, keyed by NEFF tensor name.

    The weight tensors' leading axis IS the unit axis, so the full arrays
    are already the core-concatenated layout for P("core") sharding;
    query/value are the full replicated operands.
    """
    return {
        "query": np.ascontiguousarray(query.reshape(NB, HW, C), dtype=np.float32),
        "value": np.ascontiguousarray(value.reshape(NB, HW, C), dtype=np.float32),
        "query_w": np.ascontiguousarray(query_w, dtype=np.float32),
        "key_w": np.ascontiguousarray(key_w, dtype=np.float32),
        "value_w": np.ascontiguousarray(value_w, dtype=np.float32),
    }


def gather_output(core_outs):
    """core_outs: list of [NB, UPC, 10, C] -> full [NB, 3, 3, C, 128]."""
    full = np.empty((NB, 3, 3, C, 128), dtype=np.float32)
    for i, o in enumerate(core_outs):
        norm = o[:, :, :M, :] / o[:, :, M : M + 1, :]
        # [n, u, m, c] -> [n, kh, kw, c, u]
        full[:, :, :, :, UPC * i : UPC * (i + 1)] = (
            norm.reshape(NB, UPC, 3, 3, C).transpose(0, 2, 3, 4, 1)
        )
    return full


def kernel(query, value, query_w, key_w, value_w):
    ex = _get_executor()
    jax = ex["jax"]
    g = make_global_inputs(query, value, query_w, key_w, value_w)
    # query/value go up batch-sharded (1x over the wire) and are
    # all-gathered to replicated on device; weights go up core-sharded.
    dev_in = jax.device_put(
        [g[nm] for nm in ex["in_names"]],
        [ex["core_sharding"]] * len(ex["in_names"]),
    )
    dev_in = [
        ex["replicate"](a) if nm in ex["REPL"] else a
        for nm, a in zip(ex["in_names"], dev_in)
    ]
    outs = ex["sharded"](*dev_in, *ex["zo_dev"])
    shape, _ = ex["out_shapes"][0]
    o0 = np.asarray(outs[0]).reshape(N_CORES, *shape)
    return gather_output(list(o0))

